# revision 28
# baseline (speedup 1.0000x reference)
"""Causal self-attention (B=4, S=2048, D=1024, H=16) on 8 Trainium2 cores. v2

Sharding: core c -> (batch b=c//2, head-half g=c%2, heads g*8..g*8+8).
Each core: QKV projection for its 512 q/k/v columns, causal attention for
its 8 heads, partial output projection (512 rows of w_proj). Host sums the
two partials per batch + b_proj.

v2 structure (vs v1): window-outer loop (W=512 q-windows, head-pairs inner),
joint per-si score tile [128, 1024] holding both heads (one exp instruction
per si on ScalarE), score matmuls pair-concurrent via PE row-tiles (partition
offsets 0/64), PE-broadcast normalize (K=1 matmul) instead of a DRAM
round-trip, and QKV/proj work emitted incrementally as PE fillers inside the
attention si-streams so the PE queue never waits on ScalarE.

PSUM budget (8 banks): scs [128,1024] x2 bufs = 4, ctx [65,512] x2 = 2,
fill [128,512] x2 = 2.
"""
import os
os.environ.setdefault("BASS_NEVER_TRACE", "1")

import numpy as np
import ml_dtypes

import concourse.tile as tile
from concourse import bacc, mybir
from concourse.bass_utils import run_bass_kernel_spmd

bf16 = ml_dtypes.bfloat16
FP32 = mybir.dt.float32
BF16 = mybir.dt.bfloat16
EXP = mybir.ActivationFunctionType.Exp

B, S, D = 4, 2048, 1024
H, HD = 16, 64
NCORE = 8
NH = 8            # heads per core
W = 512           # q-window
NW = S // W       # 4 windows
KT = 8            # k-tiles of D for qkv chains
SCALE = 1.0 / np.sqrt(HD)

_NC_CACHE = {}


def build_nc(reps=1, with_bias=True, phases=("qkv", "attn", "proj")):
    nc = bacc.Bacc("TRN2", target_bir_lowering=False, debug=False)
    inpT = nc.dram_tensor("inpT", [D, S], BF16, kind="ExternalInput").ap()
    wqk = nc.dram_tensor("wqk", [D, 1024], BF16, kind="ExternalInput").ap()
    wv = nc.dram_tensor("wv", [D, 512], BF16, kind="ExternalInput").ap()
    wproj = nc.dram_tensor("wproj", [512, D], BF16, kind="ExternalInput").ap()
    if with_bias:
        bqk = nc.dram_tensor("bqk", [128, 8], FP32, kind="ExternalInput").ap()
        bv = nc.dram_tensor("bv", [1, 512], BF16, kind="ExternalInput").ap()
    trimask = nc.dram_tensor("trimask", [128, 128], BF16, kind="ExternalInput").ap()
    out = nc.dram_tensor("out", [S, D], FP32, kind="ExternalOutput").ap()

    with tile.TileContext(nc) as tc:
        with (
            tc.tile_pool(name="const", bufs=1) as const,
            tc.tile_pool(name="work", bufs=1) as work,
            tc.tile_pool(name="exps", bufs=4) as expp,
            tc.tile_pool(name="small", bufs=2) as small,
            tc.tile_pool(name="outp", bufs=3) as outp,
            tc.tile_pool(name="ps", bufs=2, space="PSUM") as ps,
        ):
            inpT_sb = const.tile([128, KT, S], BF16, tag="inpT")
            nc.sync.dma_start(inpT_sb, inpT.rearrange("(t p) s -> p t s", p=128))
            wqk_sb = const.tile([128, KT, 1024], BF16, tag="wqk")
            nc.sync.dma_start(wqk_sb, wqk.rearrange("(t p) c -> p t c", p=128))
            wv_sb = const.tile([128, KT, 512], BF16, tag="wv")
            nc.sync.dma_start(wv_sb, wv.rearrange("(t p) c -> p t c", p=128))
            wproj_sb = const.tile([128, 4, 1024], BF16, tag="wproj")
            nc.sync.dma_start(wproj_sb, wproj.rearrange("(t p) e -> p t e", p=128))
            mask2_sb_t = const.tile([128, 2, 512], BF16, tag="mask2")
            for _hi in range(2):
                for _d in range(4):
                    nc.sync.dma_start(
                        mask2_sb_t[:, _hi, 128*_d:128*(_d+1)], trimask)
            ones_bf = const.tile([1, 128], BF16, tag="ones_bf")
            nc.vector.memset(ones_bf, 1.0)
            if with_bias:
                bqk_sb = const.tile([128, 8], FP32, tag="bqk")
                nc.sync.dma_start(bqk_sb, bqk)
                bv_sb = const.tile([1, 512], BF16, tag="bv")
                nc.sync.dma_start(bv_sb, bv)
            else:
                bqk_sb = bv_sb = None

            cfg = dict(nc=nc, ps=ps, expp=expp, small=small, outp=outp,
                       inpT_sb=inpT_sb, wqk_sb=wqk_sb, wv_sb=wv_sb,
                       wproj_sb=wproj_sb, bqk_sb=bqk_sb, bv_sb=bv_sb,
                       mask2_sb=mask2_sb_t, ones_bf=ones_bf,
                       out=out, with_bias=with_bias, phases=phases)

            def emit_body():
                _emit_body(work=work, **cfg)

            if reps == 1:
                emit_body()
            else:
                with tc.For_i(0, reps, 1):
                    emit_body()
    nc.compile()
    return nc


def _emit_body(nc, ps, expp, small, outp, work, inpT_sb, wqk_sb, wv_sb,
               wproj_sb, bqk_sb, bv_sb, mask2_sb, ones_bf, out,
               with_bias, phases):
    qkT_sb = work.tile([128, 8, S], BF16, tag="qkT")
    vp_sb = work.tile([128, 16, 520], BF16, tag="vp")
    ctxT_sb = work.tile([128, 4, S], BF16, tag="ctxT")
    if "qkv" not in phases and any(p.startswith("attn") for p in phases):
        # profiling-only: attn without qkv needs initialized q/k/v tiles
        nc.vector.memset(qkT_sb, 0.01)
        nc.vector.memset(vp_sb, 0.01)
    if "attn" not in phases and "proj" in phases and \
            not any(p.startswith("attn") for p in phases):
        nc.vector.memset(ctxT_sb, 0.01)
    else:
        # v' ones rows (denominator trick), written once per rep
        nc.vector.memset(vp_sb[:, :, 64::65], 1.0)
    # manually-rotated score psum: slot pair (2si%4, 2si%4+1) holds both
    # heads of step si, adjacent so ONE exp instruction covers both.
    scs_big = ps.tile([128, 4, 512], FP32, tag="scs", bufs=1,
                      name="scs_big")

    do_qkv = "qkv" in phases
    do_attn = False
    attn_mode = "full"
    for p in phases:
        if p.startswith("attn"):
            do_attn = True
            attn_mode = p.split(":")[1] if ":" in p else "full"
    do_proj = "proj" in phases

    # ---------------- filler units (micro-closure lists) ----------------
    # Each unit is a list of closures: one per matmul (or pair of matmuls)
    # plus a consumer closure, so the scheduler can interleave filler work
    # into the attention stream at matmul granularity.
    def qk_unit(ct, ch):
        """8-matmul K-chain producing qkT[:, ct, 512*ch:512*(ch+1)]."""
        cell = {}

        def mm(kt):
            def f():
                if kt == 0:
                    cell["ps"] = ps.tile([128, 512], FP32, tag="fill", bufs=2,
                                         name=f"qkps_{ct}_{ch}")
                nc.tensor.matmul(
                    cell["ps"], wqk_sb[:, kt, 128*ct:128*(ct+1)],
                    inpT_sb[:, kt, 512*ch:512*ch+512],
                    start=(kt == 0), stop=(kt == KT - 1),
                    skip_group_check=True)
            return f

        def fin():
            dst = qkT_sb[:, ct, 512*ch:512*ch+512]
            if with_bias:
                nc.vector.tensor_scalar_add(dst, cell["ps"], bqk_sb[:, ct:ct+1])
            else:
                nc.vector.tensor_copy(dst, cell["ps"])
        return [mm(kt) for kt in range(KT)] + [fin]

    def vp_unit(tt):
        """V matmul chain + strided copy into v' for one 128-token tile."""
        cell = {}

        def mm(kt):
            def f():
                if kt == 0:
                    cell["ps"] = ps.tile([128, 512], FP32, tag="fill", bufs=2,
                                         name=f"vps_{tt}")
                nc.tensor.matmul(
                    cell["ps"], inpT_sb[:, kt, 128*tt:128*(tt+1)],
                    wv_sb[:, kt, :],
                    start=(kt == 0), stop=(not with_bias and kt == KT - 1),
                    skip_group_check=True)
                if with_bias and kt == KT - 1:
                    nc.tensor.matmul(cell["ps"], ones_bf, bv_sb, start=False,
                                     stop=True, skip_group_check=True)
            return f

        def fin():
            vp_view = vp_sb[:, tt, :].rearrange(
                "p (h c) -> p h c", c=65)[:, :, 0:64]
            nc.vector.tensor_copy(vp_view,
                                  cell["ps"].rearrange("p (h c) -> p h c", c=64))
        return [mm(kt) for kt in range(KT)] + [fin]

    def proj_unit(tt, ec, eng_i):
        """Output projection for one (token tile, 512-col half)."""
        cell = {}

        def mm(kt):
            def f():
                if kt == 0:
                    cell["ps"] = ps.tile([128, 512], FP32, tag="fill", bufs=2,
                                         name=f"prps_{tt}_{ec}")
                nc.tensor.matmul(
                    cell["ps"], ctxT_sb[:, kt, 128*tt:128*(tt+1)],
                    wproj_sb[:, kt, 512*ec:512*(ec+1)],
                    start=(kt == 0), stop=(kt == 3), skip_group_check=True)
            return f

        def fin():
            o_sb = outp.tile([128, 512], FP32, tag="o", name=f"osb_{tt}_{ec}")
            nc.vector.tensor_copy(o_sb, cell["ps"])
            nc.gpsimd.dma_start(out[128*tt:128*(tt+1), 512*ec:512*(ec+1)], o_sb)
        return [fin]  # placeholder; real list built below

    def proj_unit_full(tt, ec, eng_i):
        u = proj_unit_mms(tt, ec)
        return u

    def proj_unit_mms(tt, ec):
        cell = {}

        def mm(kt):
            def f():
                if kt == 0:
                    cell["ps"] = ps.tile([128, 512], FP32, tag="fill", bufs=2,
                                         name=f"prps_{tt}_{ec}")
                nc.tensor.matmul(
                    cell["ps"], ctxT_sb[:, kt, 128*tt:128*(tt+1)],
                    wproj_sb[:, kt, 512*ec:512*(ec+1)],
                    start=(kt == 0), stop=(kt == 3), skip_group_check=True)
            return f

        def fin():
            o_sb = outp.tile([128, 512], FP32, tag="o", name=f"osb_{tt}_{ec}")
            nc.vector.tensor_copy(o_sb, cell["ps"])
            nc.gpsimd.dma_start(out[128*tt:128*(tt+1), 512*ec:512*(ec+1)], o_sb)
        return [mm(kt) for kt in range(4)] + [fin]

    # ---------------- attention pair-window ----------------
    def attn_pair_window(w, hp, fq, pending_norm, jit_vp,
                         steps_done, total_steps):
        step_ctr = steps_done
        """Window w (q cols 512w..512w+512) for head pair hp."""
        q0 = W * w
        ctx2 = None
        if attn_mode == "full":
            ctx2 = ps.tile([65, 2, W], FP32, tag="ctx", bufs=1,
                           name=f"ctx_{w}_{hp}")
        pv_queue = []

        # Step list: (regions, bank_w, masked); region = (kt, qlo, qhi, blo)
        # in window coords. Full steps carry one 512-wide k-tile. The
        # diagonal 512x512 block is packed into 3 steps: A = the two
        # disjoint clean rectangles (k-tile 4w over q[128:512] + k-tile
        # 4w+2 over q[384:512]), B = the remaining rectangle (k-tile 4w+1
        # over q[256:512]), D = all four masked diagonal 128x128 tiles in
        # one bank (one exp + ONE mask-mul for the whole band).
        b0 = 4 * w
        # Diagonal block FIRST: its exp->mask->PV chain then hides under
        # the pair's full steps instead of serializing at the pair end.
        steps = [([(b0, 128, 512, 0), (b0 + 2, 384, 512, 384)], 512, False),
                 ([(b0 + 1, 256, 512, 0)], 256, False),
                 ([(b0 + d, 128 * d, 128 * (d + 1), 128 * d)
                   for d in range(4)], 512, True)]
        steps += [([(si, 0, 512, 0)], 512, False) for si in range(4 * w)]
        n_steps = len(steps)
        pv_first = [True]

        def make_pv(regions, exs, last):
            def f():
                # start=True on the pair's first PV MM clears has_written
                # BANK-WIDE; all later MMs use start=False (bit set ->
                # accumulate, bit unset -> overwrite).
                first = pv_first[0]
                pv_first[0] = False
                for hi in range(2):
                    rhs0 = 512 * hi
                    for ri, (kt, qlo, qhi, blo) in enumerate(regions):
                        nc.tensor.matmul(
                            ctx2[:, hi, qlo:qhi],
                            vp_sb[:, kt, 65*(2*hp+hi):65*(2*hp+hi)+65],
                            exs[:, rhs0+blo:rhs0+blo+(qhi-qlo)],
                            start=(first and ri == 0),
                            stop=(last and ri == len(regions) - 1),
                            skip_group_check=True)
            return f

        for si, (regions, bank_w, masked) in enumerate(steps):
            # run pending normalize early (frees ctx psum for this pair)
            if si == 0 and pending_norm is not None:
                pending_norm[0]()
            if si == 1 and pending_norm is not None:
                pending_norm[1]()
                pending_norm = None
            # PE-queue order: PV + fillers first, score MMs last, so the
            # score MM's wait on a rotation slot doesn't head-of-line-block
            # work whose deps are already met.
            if len(pv_queue) >= 3:
                pv_queue.pop(0)()
            if jit_vp is not None:
                for j in range(len(jit_vp)):
                    if min((j * n_steps) // len(jit_vp), n_steps - 1) == si:
                        for f in jit_vp[j]:
                            f()
            fq.emit_smooth(total_steps - steps_done[0] - 1)
            # score matmuls into adjacent banks of the rotating score psum
            # (PE row-tiles 0/64 overlap across the head pair). Keep this
            # MM->exp loop minimal: masking happens on the exp OUTPUT (DVE),
            # off the rotation-critical path, and only on D steps.
            slot = (2 * step_ctr[0]) % 4
            for hi in range(2):
                po = 64 * hi
                for (kt, qlo, qhi, blo) in regions:
                    # start=True is region-scoped: overwrite + set
                    # has_written for exactly this region's elements.
                    nc.tensor.matmul(
                        scs_big[:, slot + hi, blo:blo+(qhi-qlo)],
                        qkT_sb[po:po+64, 4+hp, 128*kt:128*(kt+1)],
                        qkT_sb[po:po+64, hp, q0+qlo:q0+qhi],
                        start=True, stop=True, skip_group_check=True)
            exs = None
            if attn_mode != "sc":
                exs = expp.tile([128, 1024], BF16, tag="ex", bufs=8,
                                name=f"ex_{w}_{hp}_{si}")
                ex3 = exs.rearrange("p (h c) -> p h c", c=512)
                nc.scalar.activation(ex3[:, :, 0:bank_w],
                                     scs_big[:, slot:slot+2, 0:bank_w], EXP,
                                     scale=float(SCALE))
                if masked:
                    nc.gpsimd.tensor_mul(ex3, ex3, mask2_sb)
            if attn_mode == "full":
                pv_queue.append(make_pv(regions, exs, si == n_steps - 1))
            steps_done[0] += 1
        # drain remaining PVs back-to-back so the evacuation copy directly
        # follows the last PV in the queues.
        while pv_queue:
            pv_queue.pop(0)()

        if attn_mode != "full":
            def noop():
                pass
            return (noop, noop)

        cell = {}

        def norm_a():
            # ONE evacuation copy frees the ctx psum bank pair after a
            # single cross-engine hop; recip/broadcast/normalize then run
            # from SBUF off the rotation-critical path.
            evac = small.tile([65, 2, W], BF16, tag="evac",
                              name=f"evac_{w}_{hp}")
            with nc.allow_low_precision(reason="bf16 ctx evac"):
                nc.vector.tensor_copy(evac, ctx2)
            recipbf = small.tile([1, 2, W], BF16, tag="recipbf",
                                 name=f"rbf_{w}_{hp}")
            with nc.allow_low_precision(reason="bf16 recip for normalize"):
                nc.vector.reciprocal(recipbf, evac[64:65, :, :])
            bcb = small.tile([64, 2, W], BF16, tag="bcb",
                             name=f"bcb_{w}_{hp}")
            nc.gpsimd.partition_broadcast(bcb, recipbf)
            cell["bcb"] = bcb
            cell["evac"] = evac

        norm_a()

        def noop_a():
            pass

        def norm_b():
            for hi in range(2):
                po = 64 * hi
                nc.vector.tensor_mul(ctxT_sb[po:po+64, hp, q0:q0+W],
                                     cell["evac"][0:64, hi, :],
                                     cell["bcb"][:, hi, :])
        return (noop_a, norm_b)

    class FillQueue:
        """Deadline-ordered filler closures, smoothed across all si steps."""

        def __init__(self):
            self.items = []          # (deadline_index, closure)
            self.cursor = 0
            self.total_steps = 0

        def add(self, deadline, closures):
            for c in closures:
                self.items.append((deadline, c))

        def drain_due(self, now):
            while self.cursor < len(self.items) and \
                    self.items[self.cursor][0] <= now:
                self.items[self.cursor][1]()
                self.cursor += 1

        def emit_smooth(self, steps_left):
            remaining = len(self.items) - self.cursor
            if remaining <= 0 or steps_left <= 0:
                return
            quota = (remaining + steps_left - 1) // steps_left
            for _ in range(quota):
                if self.cursor >= len(self.items):
                    break
                self.items[self.cursor][1]()
                self.cursor += 1

    # ---------------- schedule ----------------
    if not do_attn:
        if do_qkv:
            for ct in range(8):
                for ch in range(4):
                    for f in qk_unit(ct, ch):
                        f()
            for tt in range(16):
                for f in vp_unit(tt):
                    f()
        if do_proj:
            for tt in range(16):
                for ec in range(2):
                    for f in proj_unit_mms(tt, ec):
                        f()
        return

    # upfront: q+k chunk-0 units for pair 0
    if do_qkv:
        for f in qk_unit(0, 0) + qk_unit(4, 0):
            f()

    def projs(tt_lo, tt_hi):
        us = []
        for tt in range(tt_lo, tt_hi):
            for ec in range(2):
                us += proj_unit_mms(tt, ec)
        return us

    # Global filler queue. Deadlines are pair-slot indices (w*4 + hp):
    # a closure with deadline d must run before slot d's si-loop starts.
    fq = FillQueue()
    slot = lambda w, hp: 4 * w + hp
    items = []
    if do_qkv:
        for w in range(NW):
            for hp in range(4):
                # qk chunks consumed by the NEXT pair-slot
                if hp < 3:
                    items.append((slot(w, hp + 1),
                                  qk_unit(hp + 1, w) + qk_unit(4 + hp + 1, w)))
                elif w < NW - 1:
                    items.append((slot(w + 1, 0),
                                  qk_unit(0, w + 1) + qk_unit(4, w + 1)))
        for w in range(1, NW):
            # v' tiles consumed from pair (w,0)'s early diag steps
            items.append((slot(w, 0), vp_unit(4 * w) + vp_unit(4 * w + 1)))
            items.append((slot(w, 0), vp_unit(4 * w + 2) + vp_unit(4 * w + 3)))
    items.sort(key=lambda t: t[0])
    for _d, _cs in items:
        fq.add(_d, _cs)
    if do_proj:
        # proj for window w-1 due by end of schedule; deadline large but
        # ordered after window w-1's last norm (enforced by inserting with
        # deadline slot(w,1): its closures cannot run before insertion order
        # anyway since the queue is drained in order).
        for w in range(1, NW):
            fq.add(slot(w, 1) + 100, projs(4 * (w - 1), 4 * w))

    # total si steps for smoothing
    total_steps = sum((4 * w + 3) for w in range(NW) for _ in range(4))
    steps_done = [0]
    step_ctr = steps_done  # alias: per-si global counter drives scs slots

    pending_norm = None
    for w in range(NW):
        for hp in range(4):
            fq.drain_due(slot(w, hp))
            jv = None
            if do_qkv and hp == 0 and w == 0:
                jv = [vp_unit(j) for j in range(4)]
            pending_norm = attn_pair_window(
                w, hp, fq, pending_norm, jv,
                steps_done, total_steps)
    if pending_norm is not None:
        pending_norm[0]()
        pending_norm[1]()
    while fq.cursor < len(fq.items):
        fq.items[fq.cursor][1]()
        fq.cursor += 1
    if do_proj:
        for u in projs(4 * (NW - 1), 4 * NW):
            u()


def _prep_core_inputs(core, inp, w_attn, b_attn, w_proj):
    b, g = core // 2, core % 2
    qc, kc, vc = 512 * g, D + 512 * g, 2 * D + 512 * g
    return dict(
        inpT=np.ascontiguousarray(inp[b].T).astype(bf16),
        wqk=np.concatenate(
            [w_attn[:, qc:qc+512], w_attn[:, kc:kc+512]], axis=1).astype(bf16),
        wv=w_attn[:, vc:vc+512].astype(bf16),
        wproj=np.ascontiguousarray(w_proj[512*g:512*(g+1), :]).astype(bf16),
        bqk=np.concatenate([b_attn[qc:qc+512], b_attn[kc:kc+512]])
            .astype(np.float32).reshape(8, 128).T.copy(),
        bv=b_attn[vc:vc+512].astype(bf16).reshape(1, 512),
        trimask=np.triu(np.ones((128, 128), np.float32)).astype(bf16),
    )


def kernel(inp, w_attn, b_attn, w_proj, b_proj, _results_out=None):
    inp = np.asarray(inp, dtype=np.float32)
    w_attn = np.asarray(w_attn, dtype=np.float32)
    b_attn = np.asarray(b_attn, dtype=np.float32)
    w_proj = np.asarray(w_proj, dtype=np.float32)
    b_proj = np.asarray(b_proj, dtype=np.float32)

    with_bias = bool(np.any(b_attn != 0.0))
    key = (1, with_bias)
    if key not in _NC_CACHE:
        _NC_CACHE[key] = build_nc(reps=1, with_bias=with_bias)
    nc = _NC_CACHE[key]

    in_maps = [_prep_core_inputs(c, inp, w_attn, b_attn, w_proj)
               for c in range(NCORE)]
    declared = set()
    for alloc in nc.m.functions[0].allocations:
        if isinstance(alloc, mybir.MemoryLocationSet) and alloc.kind == "ExternalInput":
            declared.add(alloc.memorylocations[0].name)
    in_maps = [{k: v for k, v in m.items() if k in declared} for m in in_maps]

    res = run_bass_kernel_spmd(nc, in_maps, core_ids=list(range(NCORE)))
    if _results_out is not None:
        _results_out.append(res)

    out = np.empty((B, S, D), np.float32)
    for b in range(B):
        out[b] = (res.results[2*b]["out"] + res.results[2*b+1]["out"]
                  + b_proj[None, :])
    return out



# revision 29
# speedup vs baseline: 1.4346x; 1.4346x over previous
"""Causal self-attention (B=4, S=2048, D=1024, H=16) on 8 Trainium2 cores. v2

Sharding: core c -> (batch b=c//2, head-half g=c%2, heads g*8..g*8+8).
Each core: QKV projection for its 512 q/k/v columns, causal attention for
its 8 heads, partial output projection (512 rows of w_proj). Host sums the
two partials per batch + b_proj.

v2 structure (vs v1): window-outer loop (W=512 q-windows, head-pairs inner),
joint per-si score tile [128, 1024] holding both heads (one exp instruction
per si on ScalarE), score matmuls pair-concurrent via PE row-tiles (partition
offsets 0/64), PE-broadcast normalize (K=1 matmul) instead of a DRAM
round-trip, and QKV/proj work emitted incrementally as PE fillers inside the
attention si-streams so the PE queue never waits on ScalarE.

PSUM budget (8 banks): scs [128,1024] x2 bufs = 4, ctx [65,512] x2 = 2,
fill [128,512] x2 = 2.
"""
import os
os.environ.setdefault("BASS_NEVER_TRACE", "1")

import numpy as np
import ml_dtypes

import concourse.tile as tile
from concourse import bacc, mybir
from concourse.bass_utils import run_bass_kernel_spmd

bf16 = ml_dtypes.bfloat16
FP32 = mybir.dt.float32
BF16 = mybir.dt.bfloat16
EXP = mybir.ActivationFunctionType.Exp

B, S, D = 4, 2048, 1024
H, HD = 16, 64
NCORE = 8
NH = 8            # heads per core
W = 512           # q-window
NW = S // W       # 4 windows
KT = 8            # k-tiles of D for qkv chains
SCALE = 1.0 / np.sqrt(HD)

_NC_CACHE = {}


def build_nc(reps=1, with_bias=True, phases=("qkv", "attn", "proj")):
    nc = bacc.Bacc("TRN2", target_bir_lowering=False, debug=False)
    inpT = nc.dram_tensor("inpT", [D, S], BF16, kind="ExternalInput").ap()
    wqk = nc.dram_tensor("wqk", [D, 1024], BF16, kind="ExternalInput").ap()
    wv = nc.dram_tensor("wv", [D, 512], BF16, kind="ExternalInput").ap()
    wproj = nc.dram_tensor("wproj", [512, D], BF16, kind="ExternalInput").ap()
    if with_bias:
        bqk = nc.dram_tensor("bqk", [128, 8], FP32, kind="ExternalInput").ap()
        bv = nc.dram_tensor("bv", [1, 512], BF16, kind="ExternalInput").ap()
    trimask = nc.dram_tensor("trimask", [128, 128], BF16, kind="ExternalInput").ap()
    out = nc.dram_tensor("out", [S, D], FP32, kind="ExternalOutput").ap()

    with tile.TileContext(nc) as tc:
        with (
            tc.tile_pool(name="const", bufs=1) as const,
            tc.tile_pool(name="work", bufs=1) as work,
            tc.tile_pool(name="exps", bufs=4) as expp,
            tc.tile_pool(name="small", bufs=2) as small,
            tc.tile_pool(name="outp", bufs=3) as outp,
            tc.tile_pool(name="ps", bufs=2, space="PSUM") as ps,
        ):
            inpT_sb = const.tile([128, KT, S], BF16, tag="inpT")
            nc.sync.dma_start(inpT_sb, inpT.rearrange("(t p) s -> p t s", p=128))
            wqk_sb = const.tile([128, KT, 1024], BF16, tag="wqk")
            nc.sync.dma_start(wqk_sb, wqk.rearrange("(t p) c -> p t c", p=128))
            wv_sb = const.tile([128, KT, 512], BF16, tag="wv")
            nc.sync.dma_start(wv_sb, wv.rearrange("(t p) c -> p t c", p=128))
            wproj_sb = const.tile([128, 4, 1024], BF16, tag="wproj")
            nc.sync.dma_start(wproj_sb, wproj.rearrange("(t p) e -> p t e", p=128))
            mask2_sb_t = const.tile([128, 2, 512], BF16, tag="mask2")
            for _hi in range(2):
                for _d in range(4):
                    nc.sync.dma_start(
                        mask2_sb_t[:, _hi, 128*_d:128*(_d+1)], trimask)
            ones_bf = const.tile([1, 128], BF16, tag="ones_bf")
            nc.vector.memset(ones_bf, 1.0)
            if with_bias:
                bqk_sb = const.tile([128, 8], FP32, tag="bqk")
                nc.sync.dma_start(bqk_sb, bqk)
                bv_sb = const.tile([1, 512], BF16, tag="bv")
                nc.sync.dma_start(bv_sb, bv)
            else:
                bqk_sb = bv_sb = None

            cfg = dict(nc=nc, ps=ps, expp=expp, small=small, outp=outp,
                       inpT_sb=inpT_sb, wqk_sb=wqk_sb, wv_sb=wv_sb,
                       wproj_sb=wproj_sb, bqk_sb=bqk_sb, bv_sb=bv_sb,
                       mask2_sb=mask2_sb_t, ones_bf=ones_bf,
                       out=out, with_bias=with_bias, phases=phases)

            def emit_body():
                _emit_body(work=work, **cfg)

            if reps == 1:
                emit_body()
            else:
                with tc.For_i(0, reps, 1):
                    emit_body()
    nc.compile()
    return nc


def _emit_body(nc, ps, expp, small, outp, work, inpT_sb, wqk_sb, wv_sb,
               wproj_sb, bqk_sb, bv_sb, mask2_sb, ones_bf, out,
               with_bias, phases):
    qkT_sb = work.tile([128, 8, S], BF16, tag="qkT")
    vp_sb = work.tile([128, 16, 520], BF16, tag="vp")
    ctxT_sb = work.tile([128, 4, S], BF16, tag="ctxT")
    if "qkv" not in phases and any(p.startswith("attn") for p in phases):
        # profiling-only: attn without qkv needs initialized q/k/v tiles
        nc.vector.memset(qkT_sb, 0.01)
        nc.vector.memset(vp_sb, 0.01)
    if "attn" not in phases and "proj" in phases and \
            not any(p.startswith("attn") for p in phases):
        nc.vector.memset(ctxT_sb, 0.01)
    else:
        # v' ones rows (denominator trick), written once per rep
        nc.vector.memset(vp_sb[:, :, 64::65], 1.0)
    # manually-rotated score psum: slot pair (2si%4, 2si%4+1) holds both
    # heads of step si, adjacent so ONE exp instruction covers both.
    scs_big = ps.tile([128, 4, 512], FP32, tag="scs", bufs=1,
                      name="scs_big")

    do_qkv = "qkv" in phases
    do_attn = False
    attn_mode = "full"
    for p in phases:
        if p.startswith("attn"):
            do_attn = True
            attn_mode = p.split(":")[1] if ":" in p else "full"
    do_proj = "proj" in phases

    # ---------------- filler units (micro-closure lists) ----------------
    # Each unit is a list of closures: one per matmul (or pair of matmuls)
    # plus a consumer closure, so the scheduler can interleave filler work
    # into the attention stream at matmul granularity.
    def qk_unit(ct, ch):
        """8-matmul K-chain producing qkT[:, ct, 512*ch:512*(ch+1)]."""
        cell = {}

        def mm(kt):
            def f():
                if kt == 0:
                    cell["ps"] = ps.tile([128, 512], FP32, tag="fill", bufs=2,
                                         name=f"qkps_{ct}_{ch}")
                nc.tensor.matmul(
                    cell["ps"], wqk_sb[:, kt, 128*ct:128*(ct+1)],
                    inpT_sb[:, kt, 512*ch:512*ch+512],
                    start=(kt == 0), stop=(kt == KT - 1),
                    skip_group_check=True)
            return f

        def fin():
            dst = qkT_sb[:, ct, 512*ch:512*ch+512]
            if with_bias:
                nc.vector.tensor_scalar_add(dst, cell["ps"], bqk_sb[:, ct:ct+1])
            else:
                nc.vector.tensor_copy(dst, cell["ps"])
        return [mm(kt) for kt in range(KT)] + [fin]

    def vp_unit(tt):
        """V matmul chain + strided copy into v' for one 128-token tile."""
        cell = {}

        def mm(kt):
            def f():
                if kt == 0:
                    cell["ps"] = ps.tile([128, 512], FP32, tag="fill", bufs=2,
                                         name=f"vps_{tt}")
                nc.tensor.matmul(
                    cell["ps"], inpT_sb[:, kt, 128*tt:128*(tt+1)],
                    wv_sb[:, kt, :],
                    start=(kt == 0), stop=(not with_bias and kt == KT - 1),
                    skip_group_check=True)
                if with_bias and kt == KT - 1:
                    nc.tensor.matmul(cell["ps"], ones_bf, bv_sb, start=False,
                                     stop=True, skip_group_check=True)
            return f

        def fin():
            vp_view = vp_sb[:, tt, :].rearrange(
                "p (h c) -> p h c", c=65)[:, :, 0:64]
            nc.vector.tensor_copy(vp_view,
                                  cell["ps"].rearrange("p (h c) -> p h c", c=64))
        return [mm(kt) for kt in range(KT)] + [fin]

    def proj_unit(tt, ec, eng_i):
        """Output projection for one (token tile, 512-col half)."""
        cell = {}

        def mm(kt):
            def f():
                if kt == 0:
                    cell["ps"] = ps.tile([128, 512], FP32, tag="fill", bufs=2,
                                         name=f"prps_{tt}_{ec}")
                nc.tensor.matmul(
                    cell["ps"], ctxT_sb[:, kt, 128*tt:128*(tt+1)],
                    wproj_sb[:, kt, 512*ec:512*(ec+1)],
                    start=(kt == 0), stop=(kt == 3), skip_group_check=True)
            return f

        def fin():
            o_sb = outp.tile([128, 512], FP32, tag="o", name=f"osb_{tt}_{ec}")
            nc.vector.tensor_copy(o_sb, cell["ps"])
            nc.gpsimd.dma_start(out[128*tt:128*(tt+1), 512*ec:512*(ec+1)], o_sb)
        return [fin]  # placeholder; real list built below

    def proj_unit_full(tt, ec, eng_i):
        u = proj_unit_mms(tt, ec)
        return u

    def proj_unit_mms(tt, ec):
        cell = {}

        def mm(kt):
            def f():
                if kt == 0:
                    cell["ps"] = ps.tile([128, 512], FP32, tag="fill", bufs=2,
                                         name=f"prps_{tt}_{ec}")
                nc.tensor.matmul(
                    cell["ps"], ctxT_sb[:, kt, 128*tt:128*(tt+1)],
                    wproj_sb[:, kt, 512*ec:512*(ec+1)],
                    start=(kt == 0), stop=(kt == 3), skip_group_check=True)
            return f

        def fin():
            o_sb = outp.tile([128, 512], FP32, tag="o", name=f"osb_{tt}_{ec}")
            nc.vector.tensor_copy(o_sb, cell["ps"])
            nc.gpsimd.dma_start(out[128*tt:128*(tt+1), 512*ec:512*(ec+1)], o_sb)
        return [mm(kt) for kt in range(4)] + [fin]

    # ---------------- attention pair-window ----------------
    def attn_pair_window(w, hp, fq, pending_norm, jit_vp,
                         steps_done, total_steps):
        step_ctr = steps_done
        """Window w (q cols 512w..512w+512) for head pair hp."""
        q0 = W * w
        ctx2 = None
        if attn_mode == "full":
            ctx2 = ps.tile([65, 2, W], FP32, tag="ctx", bufs=1,
                           name=f"ctx_{w}_{hp}")
        pv_queue = []

        # Step list: (regions, bank_w, masked); region = (kt, qlo, qhi, blo)
        # in window coords. Full steps carry one 512-wide k-tile. The
        # diagonal 512x512 block is packed into 3 steps: A = the two
        # disjoint clean rectangles (k-tile 4w over q[128:512] + k-tile
        # 4w+2 over q[384:512]), B = the remaining rectangle (k-tile 4w+1
        # over q[256:512]), D = all four masked diagonal 128x128 tiles in
        # one bank (one exp + ONE mask-mul for the whole band).
        b0 = 4 * w
        # Diagonal block FIRST: its exp->mask->PV chain then hides under
        # the pair's full steps instead of serializing at the pair end.
        steps = [([(b0, 128, 512, 0), (b0 + 2, 384, 512, 384)], 512, False),
                 ([(b0 + 1, 256, 512, 0)], 256, False),
                 ([(b0 + d, 128 * d, 128 * (d + 1), 128 * d)
                   for d in range(4)], 512, True)]
        steps += [([(si, 0, 512, 0)], 512, False) for si in range(4 * w)]
        n_steps = len(steps)
        pv_first = [True]

        def make_pv(regions, exs, last):
            def f():
                # start=True on the pair's first PV MM clears has_written
                # BANK-WIDE; all later MMs use start=False (bit set ->
                # accumulate, bit unset -> overwrite).
                first = pv_first[0]
                pv_first[0] = False
                for hi in range(2):
                    rhs0 = 512 * hi
                    for ri, (kt, qlo, qhi, blo) in enumerate(regions):
                        nc.tensor.matmul(
                            ctx2[:, hi, qlo:qhi],
                            vp_sb[:, kt, 65*(2*hp+hi):65*(2*hp+hi)+65],
                            exs[:, rhs0+blo:rhs0+blo+(qhi-qlo)],
                            start=(first and ri == 0),
                            stop=(last and ri == len(regions) - 1),
                            skip_group_check=True)
            return f

        for si, (regions, bank_w, masked) in enumerate(steps):
            # run pending normalize early (frees ctx psum for this pair)
            if si == 0 and pending_norm is not None:
                pending_norm[0]()
            if si == 1 and pending_norm is not None:
                pending_norm[1]()
                pending_norm = None
            # PE-queue order: PV + fillers first, score MMs last, so the
            # score MM's wait on a rotation slot doesn't head-of-line-block
            # work whose deps are already met.
            if len(pv_queue) >= 3:
                pv_queue.pop(0)()
            if jit_vp is not None:
                for j in range(len(jit_vp)):
                    if min((j * n_steps) // len(jit_vp), n_steps - 1) == si:
                        for f in jit_vp[j]:
                            f()
            fq.emit_smooth(total_steps - steps_done[0] - 1)
            # score matmuls into adjacent banks of the rotating score psum
            # (PE row-tiles 0/64 overlap across the head pair). Keep this
            # MM->exp loop minimal: masking happens on the exp OUTPUT (DVE),
            # off the rotation-critical path, and only on D steps.
            slot = (2 * step_ctr[0]) % 4
            for hi in range(2):
                po = 64 * hi
                for (kt, qlo, qhi, blo) in regions:
                    # start=True is region-scoped: overwrite + set
                    # has_written for exactly this region's elements.
                    nc.tensor.matmul(
                        scs_big[:, slot + hi, blo:blo+(qhi-qlo)],
                        qkT_sb[po:po+64, 4+hp, 128*kt:128*(kt+1)],
                        qkT_sb[po:po+64, hp, q0+qlo:q0+qhi],
                        start=True, stop=True, skip_group_check=True)
            exs = None
            if attn_mode != "sc":
                exs = expp.tile([128, 1024], BF16, tag="ex", bufs=8,
                                name=f"ex_{w}_{hp}_{si}")
                ex3 = exs.rearrange("p (h c) -> p h c", c=512)
                nc.scalar.activation(ex3[:, :, 0:bank_w],
                                     scs_big[:, slot:slot+2, 0:bank_w], EXP,
                                     scale=float(SCALE))
                if masked:
                    nc.vector.tensor_mul(ex3, ex3, mask2_sb)
            if attn_mode == "full":
                pv_queue.append(make_pv(regions, exs, si == n_steps - 1))
            steps_done[0] += 1
        # drain remaining PVs back-to-back so the evacuation copy directly
        # follows the last PV in the queues.
        while pv_queue:
            pv_queue.pop(0)()

        if attn_mode != "full":
            def noop():
                pass
            return (noop, noop)

        cell = {}

        def norm_a():
            # ONE evacuation copy frees the ctx psum bank pair after a
            # single cross-engine hop; recip/broadcast/normalize then run
            # from SBUF off the rotation-critical path.
            evac = small.tile([65, 2, W], BF16, tag="evac",
                              name=f"evac_{w}_{hp}")
            with nc.allow_low_precision(reason="bf16 ctx evac"):
                nc.vector.tensor_copy(evac, ctx2)
            recipbf = small.tile([1, 2, W], BF16, tag="recipbf",
                                 name=f"rbf_{w}_{hp}")
            with nc.allow_low_precision(reason="bf16 recip for normalize"):
                nc.vector.reciprocal(recipbf, evac[64:65, :, :])
            bcb = small.tile([64, 2, W], BF16, tag="bcb",
                             name=f"bcb_{w}_{hp}")
            nc.gpsimd.partition_broadcast(bcb, recipbf)
            cell["bcb"] = bcb
            cell["evac"] = evac

        norm_a()

        def noop_a():
            pass

        def norm_b():
            for hi in range(2):
                po = 64 * hi
                nc.vector.tensor_mul(ctxT_sb[po:po+64, hp, q0:q0+W],
                                     cell["evac"][0:64, hi, :],
                                     cell["bcb"][:, hi, :])
        return (noop_a, norm_b)

    class FillQueue:
        """Deadline-ordered filler closures, smoothed across all si steps."""

        def __init__(self):
            self.items = []          # (deadline_index, closure)
            self.cursor = 0
            self.total_steps = 0

        def add(self, deadline, closures):
            for c in closures:
                self.items.append((deadline, c))

        def drain_due(self, now):
            while self.cursor < len(self.items) and \
                    self.items[self.cursor][0] <= now:
                self.items[self.cursor][1]()
                self.cursor += 1

        def emit_smooth(self, steps_left):
            remaining = len(self.items) - self.cursor
            if remaining <= 0 or steps_left <= 0:
                return
            quota = (remaining + steps_left - 1) // steps_left
            for _ in range(quota):
                if self.cursor >= len(self.items):
                    break
                self.items[self.cursor][1]()
                self.cursor += 1

    # ---------------- schedule ----------------
    if not do_attn:
        if do_qkv:
            for ct in range(8):
                for ch in range(4):
                    for f in qk_unit(ct, ch):
                        f()
            for tt in range(16):
                for f in vp_unit(tt):
                    f()
        if do_proj:
            for tt in range(16):
                for ec in range(2):
                    for f in proj_unit_mms(tt, ec):
                        f()
        return

    # upfront: q+k chunk-0 units for pair 0
    if do_qkv:
        for f in qk_unit(0, 0) + qk_unit(4, 0):
            f()

    def projs(tt_lo, tt_hi):
        us = []
        for tt in range(tt_lo, tt_hi):
            for ec in range(2):
                us += proj_unit_mms(tt, ec)
        return us

    # Global filler queue. Deadlines are pair-slot indices (w*4 + hp):
    # a closure with deadline d must run before slot d's si-loop starts.
    fq = FillQueue()
    slot = lambda w, hp: 4 * w + hp
    items = []
    if do_qkv:
        for w in range(NW):
            for hp in range(4):
                # qk chunks consumed by the NEXT pair-slot
                if hp < 3:
                    items.append((slot(w, hp + 1),
                                  qk_unit(hp + 1, w) + qk_unit(4 + hp + 1, w)))
                elif w < NW - 1:
                    items.append((slot(w + 1, 0),
                                  qk_unit(0, w + 1) + qk_unit(4, w + 1)))
        for w in range(1, NW):
            # v' tiles consumed from pair (w,0)'s early diag steps
            items.append((slot(w, 0), vp_unit(4 * w) + vp_unit(4 * w + 1)))
            items.append((slot(w, 0), vp_unit(4 * w + 2) + vp_unit(4 * w + 3)))
    items.sort(key=lambda t: t[0])
    for _d, _cs in items:
        fq.add(_d, _cs)
    if do_proj:
        # proj for window w-1 due by end of schedule; deadline large but
        # ordered after window w-1's last norm (enforced by inserting with
        # deadline slot(w,1): its closures cannot run before insertion order
        # anyway since the queue is drained in order).
        for w in range(1, NW):
            fq.add(slot(w, 1) + 100, projs(4 * (w - 1), 4 * w))

    # total si steps for smoothing
    total_steps = sum((4 * w + 3) for w in range(NW) for _ in range(4))
    steps_done = [0]
    step_ctr = steps_done  # alias: per-si global counter drives scs slots

    pending_norm = None
    for w in range(NW):
        for hp in range(4):
            fq.drain_due(slot(w, hp))
            jv = None
            if do_qkv and hp == 0 and w == 0:
                jv = [vp_unit(j) for j in range(4)]
            pending_norm = attn_pair_window(
                w, hp, fq, pending_norm, jv,
                steps_done, total_steps)
    if pending_norm is not None:
        pending_norm[0]()
        pending_norm[1]()
    while fq.cursor < len(fq.items):
        fq.items[fq.cursor][1]()
        fq.cursor += 1
    if do_proj:
        for u in projs(4 * (NW - 1), 4 * NW):
            u()


def _prep_core_inputs(core, inp, w_attn, b_attn, w_proj):
    b, g = core // 2, core % 2
    qc, kc, vc = 512 * g, D + 512 * g, 2 * D + 512 * g
    return dict(
        inpT=np.ascontiguousarray(inp[b].T).astype(bf16),
        wqk=np.concatenate(
            [w_attn[:, qc:qc+512], w_attn[:, kc:kc+512]], axis=1).astype(bf16),
        wv=w_attn[:, vc:vc+512].astype(bf16),
        wproj=np.ascontiguousarray(w_proj[512*g:512*(g+1), :]).astype(bf16),
        bqk=np.concatenate([b_attn[qc:qc+512], b_attn[kc:kc+512]])
            .astype(np.float32).reshape(8, 128).T.copy(),
        bv=b_attn[vc:vc+512].astype(bf16).reshape(1, 512),
        trimask=np.triu(np.ones((128, 128), np.float32)).astype(bf16),
    )


def kernel(inp, w_attn, b_attn, w_proj, b_proj, _results_out=None):
    inp = np.asarray(inp, dtype=np.float32)
    w_attn = np.asarray(w_attn, dtype=np.float32)
    b_attn = np.asarray(b_attn, dtype=np.float32)
    w_proj = np.asarray(w_proj, dtype=np.float32)
    b_proj = np.asarray(b_proj, dtype=np.float32)

    with_bias = bool(np.any(b_attn != 0.0))
    key = (1, with_bias)
    if key not in _NC_CACHE:
        _NC_CACHE[key] = build_nc(reps=1, with_bias=with_bias)
    nc = _NC_CACHE[key]

    in_maps = [_prep_core_inputs(c, inp, w_attn, b_attn, w_proj)
               for c in range(NCORE)]
    declared = set()
    for alloc in nc.m.functions[0].allocations:
        if isinstance(alloc, mybir.MemoryLocationSet) and alloc.kind == "ExternalInput":
            declared.add(alloc.memorylocations[0].name)
    in_maps = [{k: v for k, v in m.items() if k in declared} for m in in_maps]

    res = run_bass_kernel_spmd(nc, in_maps, core_ids=list(range(NCORE)))
    if _results_out is not None:
        _results_out.append(res)

    out = np.empty((B, S, D), np.float32)
    for b in range(B):
        out[b] = (res.results[2*b]["out"] + res.results[2*b+1]["out"]
                  + b_proj[None, :])
    return out



# revision 30
# speedup vs baseline: 1.4527x; 1.0126x over previous
"""Causal self-attention (B=4, S=2048, D=1024, H=16) on 8 Trainium2 cores. v2

Sharding: core c -> (batch b=c//2, head-half g=c%2, heads g*8..g*8+8).
Each core: QKV projection for its 512 q/k/v columns, causal attention for
its 8 heads, partial output projection (512 rows of w_proj). Host sums the
two partials per batch + b_proj.

v2 structure (vs v1): window-outer loop (W=512 q-windows, head-pairs inner),
joint per-si score tile [128, 1024] holding both heads (one exp instruction
per si on ScalarE), score matmuls pair-concurrent via PE row-tiles (partition
offsets 0/64), PE-broadcast normalize (K=1 matmul) instead of a DRAM
round-trip, and QKV/proj work emitted incrementally as PE fillers inside the
attention si-streams so the PE queue never waits on ScalarE.

PSUM budget (8 banks): scs [128,1024] x2 bufs = 4, ctx [65,512] x2 = 2,
fill [128,512] x2 = 2.
"""
import os
os.environ.setdefault("BASS_NEVER_TRACE", "1")

import numpy as np
import ml_dtypes

import concourse.tile as tile
from concourse import bacc, mybir
from concourse.bass_utils import run_bass_kernel_spmd

bf16 = ml_dtypes.bfloat16
FP32 = mybir.dt.float32
BF16 = mybir.dt.bfloat16
EXP = mybir.ActivationFunctionType.Exp

B, S, D = 4, 2048, 1024
H, HD = 16, 64
NCORE = 8
NH = 8            # heads per core
W = 512           # q-window
NW = S // W       # 4 windows
KT = 8            # k-tiles of D for qkv chains
SCALE = 1.0 / np.sqrt(HD)

_NC_CACHE = {}


def build_nc(reps=1, with_bias=True, phases=("qkv", "attn", "proj")):
    nc = bacc.Bacc("TRN2", target_bir_lowering=False, debug=False)
    inpT = nc.dram_tensor("inpT", [D, S], BF16, kind="ExternalInput").ap()
    wqk = nc.dram_tensor("wqk", [D, 1024], BF16, kind="ExternalInput").ap()
    wv = nc.dram_tensor("wv", [D, 512], BF16, kind="ExternalInput").ap()
    wproj = nc.dram_tensor("wproj", [512, D], BF16, kind="ExternalInput").ap()
    if with_bias:
        bqk = nc.dram_tensor("bqk", [128, 8], FP32, kind="ExternalInput").ap()
        bv = nc.dram_tensor("bv", [1, 512], BF16, kind="ExternalInput").ap()
    trimask = nc.dram_tensor("trimask", [128, 128], BF16, kind="ExternalInput").ap()
    out = nc.dram_tensor("out", [S, D], FP32, kind="ExternalOutput").ap()

    with tile.TileContext(nc) as tc:
        with (
            tc.tile_pool(name="const", bufs=1) as const,
            tc.tile_pool(name="work", bufs=1) as work,
            tc.tile_pool(name="exps", bufs=4) as expp,
            tc.tile_pool(name="small", bufs=2) as small,
            tc.tile_pool(name="outp", bufs=3) as outp,
            tc.tile_pool(name="ps", bufs=2, space="PSUM") as ps,
        ):
            inpT_sb = const.tile([128, KT, S], BF16, tag="inpT")
            nc.sync.dma_start(inpT_sb, inpT.rearrange("(t p) s -> p t s", p=128))
            wqk_sb = const.tile([128, KT, 1024], BF16, tag="wqk")
            nc.sync.dma_start(wqk_sb, wqk.rearrange("(t p) c -> p t c", p=128))
            wv_sb = const.tile([128, KT, 512], BF16, tag="wv")
            nc.sync.dma_start(wv_sb, wv.rearrange("(t p) c -> p t c", p=128))
            wproj_sb = const.tile([128, 4, 1024], BF16, tag="wproj")
            nc.sync.dma_start(wproj_sb, wproj.rearrange("(t p) e -> p t e", p=128))
            mask2_sb_t = const.tile([128, 2, 512], BF16, tag="mask2")
            for _hi in range(2):
                for _d in range(4):
                    nc.sync.dma_start(
                        mask2_sb_t[:, _hi, 128*_d:128*(_d+1)], trimask)
            ones_bf = const.tile([1, 128], BF16, tag="ones_bf")
            nc.vector.memset(ones_bf, 1.0)
            if with_bias:
                bqk_sb = const.tile([128, 8], FP32, tag="bqk")
                nc.sync.dma_start(bqk_sb, bqk)
                bv_sb = const.tile([1, 512], BF16, tag="bv")
                nc.sync.dma_start(bv_sb, bv)
            else:
                bqk_sb = bv_sb = None

            cfg = dict(nc=nc, ps=ps, expp=expp, small=small, outp=outp,
                       inpT_sb=inpT_sb, wqk_sb=wqk_sb, wv_sb=wv_sb,
                       wproj_sb=wproj_sb, bqk_sb=bqk_sb, bv_sb=bv_sb,
                       mask2_sb=mask2_sb_t, ones_bf=ones_bf,
                       out=out, with_bias=with_bias, phases=phases)

            def emit_body():
                _emit_body(work=work, **cfg)

            if reps == 1:
                emit_body()
            else:
                with tc.For_i(0, reps, 1):
                    emit_body()
    nc.compile()
    return nc


def _emit_body(nc, ps, expp, small, outp, work, inpT_sb, wqk_sb, wv_sb,
               wproj_sb, bqk_sb, bv_sb, mask2_sb, ones_bf, out,
               with_bias, phases):
    qkT_sb = work.tile([128, 8, S], BF16, tag="qkT")
    vp_sb = work.tile([128, 16, 520], BF16, tag="vp")
    ctxT_sb = work.tile([128, 4, S], BF16, tag="ctxT")
    if "qkv" not in phases and any(p.startswith("attn") for p in phases):
        # profiling-only: attn without qkv needs initialized q/k/v tiles
        nc.vector.memset(qkT_sb, 0.01)
        nc.vector.memset(vp_sb, 0.01)
    if "attn" not in phases and "proj" in phases and \
            not any(p.startswith("attn") for p in phases):
        nc.vector.memset(ctxT_sb, 0.01)
    else:
        # v' ones rows (denominator trick), written once per rep
        nc.vector.memset(vp_sb[:, :, 64::65], 1.0)
    # manually-rotated score psum: slot pair (2si%4, 2si%4+1) holds both
    # heads of step si, adjacent so ONE exp instruction covers both.
    scs_big = ps.tile([128, 4, 512], FP32, tag="scs", bufs=1,
                      name="scs_big")

    do_qkv = "qkv" in phases
    do_attn = False
    attn_mode = "full"
    for p in phases:
        if p.startswith("attn"):
            do_attn = True
            attn_mode = p.split(":")[1] if ":" in p else "full"
    do_proj = "proj" in phases

    # ---------------- filler units (micro-closure lists) ----------------
    # Each unit is a list of closures: one per matmul (or pair of matmuls)
    # plus a consumer closure, so the scheduler can interleave filler work
    # into the attention stream at matmul granularity.
    def qk_unit(ct, ch):
        """8-matmul K-chain producing qkT[:, ct, 512*ch:512*(ch+1)]."""
        cell = {}

        def mm(kt):
            def f():
                if kt == 0:
                    cell["ps"] = ps.tile([128, 512], FP32, tag="fill", bufs=2,
                                         name=f"qkps_{ct}_{ch}")
                nc.tensor.matmul(
                    cell["ps"], wqk_sb[:, kt, 128*ct:128*(ct+1)],
                    inpT_sb[:, kt, 512*ch:512*ch+512],
                    start=(kt == 0), stop=(kt == KT - 1),
                    skip_group_check=True)
            return f

        def fin():
            dst = qkT_sb[:, ct, 512*ch:512*ch+512]
            if with_bias:
                nc.vector.tensor_scalar_add(dst, cell["ps"], bqk_sb[:, ct:ct+1])
            else:
                nc.vector.tensor_copy(dst, cell["ps"])
        return [mm(kt) for kt in range(KT)] + [fin]

    def vp_unit(tt):
        """V matmul chain + strided copy into v' for one 128-token tile."""
        cell = {}

        def mm(kt):
            def f():
                if kt == 0:
                    cell["ps"] = ps.tile([128, 512], FP32, tag="fill", bufs=2,
                                         name=f"vps_{tt}")
                nc.tensor.matmul(
                    cell["ps"], inpT_sb[:, kt, 128*tt:128*(tt+1)],
                    wv_sb[:, kt, :],
                    start=(kt == 0), stop=(not with_bias and kt == KT - 1),
                    skip_group_check=True)
                if with_bias and kt == KT - 1:
                    nc.tensor.matmul(cell["ps"], ones_bf, bv_sb, start=False,
                                     stop=True, skip_group_check=True)
            return f

        def fin():
            vp_view = vp_sb[:, tt, :].rearrange(
                "p (h c) -> p h c", c=65)[:, :, 0:64]
            nc.vector.tensor_copy(vp_view,
                                  cell["ps"].rearrange("p (h c) -> p h c", c=64))
        return [mm(kt) for kt in range(KT)] + [fin]

    def proj_unit(tt, ec, eng_i):
        """Output projection for one (token tile, 512-col half)."""
        cell = {}

        def mm(kt):
            def f():
                if kt == 0:
                    cell["ps"] = ps.tile([128, 512], FP32, tag="fill", bufs=2,
                                         name=f"prps_{tt}_{ec}")
                nc.tensor.matmul(
                    cell["ps"], ctxT_sb[:, kt, 128*tt:128*(tt+1)],
                    wproj_sb[:, kt, 512*ec:512*(ec+1)],
                    start=(kt == 0), stop=(kt == 3), skip_group_check=True)
            return f

        def fin():
            o_sb = outp.tile([128, 512], FP32, tag="o", name=f"osb_{tt}_{ec}")
            nc.vector.tensor_copy(o_sb, cell["ps"])
            nc.gpsimd.dma_start(out[128*tt:128*(tt+1), 512*ec:512*(ec+1)], o_sb)
        return [fin]  # placeholder; real list built below

    def proj_unit_full(tt, ec, eng_i):
        u = proj_unit_mms(tt, ec)
        return u

    def proj_unit_mms(tt, ec):
        cell = {}

        def mm(kt):
            def f():
                if kt == 0:
                    cell["ps"] = ps.tile([128, 512], FP32, tag="fill", bufs=2,
                                         name=f"prps_{tt}_{ec}")
                nc.tensor.matmul(
                    cell["ps"], ctxT_sb[:, kt, 128*tt:128*(tt+1)],
                    wproj_sb[:, kt, 512*ec:512*(ec+1)],
                    start=(kt == 0), stop=(kt == 3), skip_group_check=True)
            return f

        def fin():
            o_sb = outp.tile([128, 512], FP32, tag="o", name=f"osb_{tt}_{ec}")
            nc.vector.tensor_copy(o_sb, cell["ps"])
            nc.gpsimd.dma_start(out[128*tt:128*(tt+1), 512*ec:512*(ec+1)], o_sb)
        return [mm(kt) for kt in range(4)] + [fin]

    # ---------------- attention pair-window ----------------
    def attn_pair_window(w, hp, fq, pending_norm, jit_vp,
                         steps_done, total_steps):
        step_ctr = steps_done
        """Window w (q cols 512w..512w+512) for head pair hp."""
        q0 = W * w
        ctx2 = None
        if attn_mode == "full":
            ctx2 = ps.tile([65, 2, W], FP32, tag="ctx", bufs=1,
                           name=f"ctx_{w}_{hp}")
        pv_queue = []

        # Step list: (regions, bank_w, masked); region = (kt, qlo, qhi, blo)
        # in window coords. Full steps carry one 512-wide k-tile. The
        # diagonal 512x512 block is packed into 3 steps: A = the two
        # disjoint clean rectangles (k-tile 4w over q[128:512] + k-tile
        # 4w+2 over q[384:512]), B = the remaining rectangle (k-tile 4w+1
        # over q[256:512]), D = all four masked diagonal 128x128 tiles in
        # one bank (one exp + ONE mask-mul for the whole band).
        b0 = 4 * w
        # Diagonal block FIRST: its exp->mask->PV chain then hides under
        # the pair's full steps instead of serializing at the pair end.
        steps = [([(b0, 128, 512, 0), (b0 + 2, 384, 512, 384)], 512, False),
                 ([(b0 + 1, 256, 512, 0)], 256, False),
                 ([(b0 + d, 128 * d, 128 * (d + 1), 128 * d)
                   for d in range(4)], 512, True)]
        steps += [([(si, 0, 512, 0)], 512, False) for si in range(4 * w)]
        n_steps = len(steps)
        pv_first = [True]

        def make_pv(regions, exs, last):
            def f():
                # start=True on the pair's first PV MM clears has_written
                # BANK-WIDE; all later MMs use start=False (bit set ->
                # accumulate, bit unset -> overwrite).
                first = pv_first[0]
                pv_first[0] = False
                for hi in range(2):
                    rhs0 = 512 * hi
                    for ri, (kt, qlo, qhi, blo) in enumerate(regions):
                        nc.tensor.matmul(
                            ctx2[:, hi, qlo:qhi],
                            vp_sb[:, kt, 65*(2*hp+hi):65*(2*hp+hi)+65],
                            exs[:, rhs0+blo:rhs0+blo+(qhi-qlo)],
                            start=(first and ri == 0),
                            stop=(last and ri == len(regions) - 1),
                            skip_group_check=True)
            return f

        for si, (regions, bank_w, masked) in enumerate(steps):
            # run pending normalize early (frees ctx psum for this pair)
            if si == 0 and pending_norm is not None:
                pending_norm[0]()
            if si == 1 and pending_norm is not None:
                pending_norm[1]()
                pending_norm = None
            # PE-queue order: PV + fillers first, score MMs last, so the
            # score MM's wait on a rotation slot doesn't head-of-line-block
            # work whose deps are already met.
            if len(pv_queue) >= 3:
                pv_queue.pop(0)()
            if jit_vp is not None:
                for j in range(len(jit_vp)):
                    if min((j * n_steps) // len(jit_vp), n_steps - 1) == si:
                        for f in jit_vp[j]:
                            f()
            fq.emit_smooth(total_steps - steps_done[0] - 1)
            # score matmuls into adjacent banks of the rotating score psum
            # (PE row-tiles 0/64 overlap across the head pair). Keep this
            # MM->exp loop minimal: masking happens on the exp OUTPUT (DVE),
            # off the rotation-critical path, and only on D steps.
            slot = (2 * step_ctr[0]) % 4
            for hi in range(2):
                po = 64 * hi
                for (kt, qlo, qhi, blo) in regions:
                    # start=True is region-scoped: overwrite + set
                    # has_written for exactly this region's elements.
                    nc.tensor.matmul(
                        scs_big[:, slot + hi, blo:blo+(qhi-qlo)],
                        qkT_sb[po:po+64, 4+hp, 128*kt:128*(kt+1)],
                        qkT_sb[po:po+64, hp, q0+qlo:q0+qhi],
                        start=True, stop=True, skip_group_check=True)
            exs = None
            if attn_mode != "sc":
                exs = expp.tile([128, 1024], BF16, tag="ex", bufs=8,
                                name=f"ex_{w}_{hp}_{si}")
                ex3 = exs.rearrange("p (h c) -> p h c", c=512)
                nc.scalar.activation(ex3[:, :, 0:bank_w],
                                     scs_big[:, slot:slot+2, 0:bank_w], EXP,
                                     scale=float(SCALE))
                if masked:
                    nc.vector.tensor_mul(ex3, ex3, mask2_sb)
            if attn_mode == "full":
                pv_queue.append(make_pv(regions, exs, si == n_steps - 1))
            steps_done[0] += 1
        # drain remaining PVs, interleaving filler to hide the last exps
        while pv_queue:
            pv_queue.pop(0)()
            fq.emit_smooth(total_steps - steps_done[0])

        if attn_mode != "full":
            def noop():
                pass
            return (noop, noop)

        cell = {}

        def norm_a():
            # ONE evacuation copy frees the ctx psum bank pair after a
            # single cross-engine hop; recip/broadcast/normalize then run
            # from SBUF off the rotation-critical path.
            evac = small.tile([65, 2, W], BF16, tag="evac",
                              name=f"evac_{w}_{hp}")
            with nc.allow_low_precision(reason="bf16 ctx evac"):
                nc.vector.tensor_copy(evac, ctx2)
            recipbf = small.tile([1, 2, W], BF16, tag="recipbf",
                                 name=f"rbf_{w}_{hp}")
            with nc.allow_low_precision(reason="bf16 recip for normalize"):
                nc.vector.reciprocal(recipbf, evac[64:65, :, :])
            bcb = small.tile([64, 2, W], BF16, tag="bcb",
                             name=f"bcb_{w}_{hp}")
            nc.gpsimd.partition_broadcast(bcb, recipbf)
            cell["bcb"] = bcb
            cell["evac"] = evac

        def norm_b():
            for hi in range(2):
                po = 64 * hi
                nc.vector.tensor_mul(ctxT_sb[po:po+64, hp, q0:q0+W],
                                     cell["evac"][0:64, hi, :],
                                     cell["bcb"][:, hi, :])
        return (norm_a, norm_b)

    class FillQueue:
        """Deadline-ordered filler closures, smoothed across all si steps."""

        def __init__(self):
            self.items = []          # (deadline_index, closure)
            self.cursor = 0
            self.total_steps = 0

        def add(self, deadline, closures):
            for c in closures:
                self.items.append((deadline, c))

        def drain_due(self, now):
            while self.cursor < len(self.items) and \
                    self.items[self.cursor][0] <= now:
                self.items[self.cursor][1]()
                self.cursor += 1

        def emit_smooth(self, steps_left):
            remaining = len(self.items) - self.cursor
            if remaining <= 0 or steps_left <= 0:
                return
            quota = (remaining + steps_left - 1) // steps_left
            for _ in range(quota):
                if self.cursor >= len(self.items):
                    break
                self.items[self.cursor][1]()
                self.cursor += 1

    # ---------------- schedule ----------------
    if not do_attn:
        if do_qkv:
            for ct in range(8):
                for ch in range(4):
                    for f in qk_unit(ct, ch):
                        f()
            for tt in range(16):
                for f in vp_unit(tt):
                    f()
        if do_proj:
            for tt in range(16):
                for ec in range(2):
                    for f in proj_unit_mms(tt, ec):
                        f()
        return

    # upfront: q+k chunk-0 units for pair 0
    if do_qkv:
        for f in qk_unit(0, 0) + qk_unit(4, 0):
            f()

    def projs(tt_lo, tt_hi):
        us = []
        for tt in range(tt_lo, tt_hi):
            for ec in range(2):
                us += proj_unit_mms(tt, ec)
        return us

    # Global filler queue. Deadlines are pair-slot indices (w*4 + hp):
    # a closure with deadline d must run before slot d's si-loop starts.
    fq = FillQueue()
    slot = lambda w, hp: 4 * w + hp
    items = []
    if do_qkv:
        for w in range(NW):
            for hp in range(4):
                # qk chunks consumed by the NEXT pair-slot
                if hp < 3:
                    items.append((slot(w, hp + 1),
                                  qk_unit(hp + 1, w) + qk_unit(4 + hp + 1, w)))
                elif w < NW - 1:
                    items.append((slot(w + 1, 0),
                                  qk_unit(0, w + 1) + qk_unit(4, w + 1)))
        for w in range(1, NW):
            # v' tiles consumed from pair (w,0)'s early diag steps
            items.append((slot(w, 0), vp_unit(4 * w) + vp_unit(4 * w + 1)))
            items.append((slot(w, 0), vp_unit(4 * w + 2) + vp_unit(4 * w + 3)))
    items.sort(key=lambda t: t[0])
    for _d, _cs in items:
        fq.add(_d, _cs)
    if do_proj:
        # proj for window w-1 due by end of schedule; deadline large but
        # ordered after window w-1's last norm (enforced by inserting with
        # deadline slot(w,1): its closures cannot run before insertion order
        # anyway since the queue is drained in order).
        for w in range(1, NW):
            fq.add(slot(w, 1) + 100, projs(4 * (w - 1), 4 * w))

    # total si steps for smoothing
    total_steps = sum((4 * w + 3) for w in range(NW) for _ in range(4))
    steps_done = [0]
    step_ctr = steps_done  # alias: per-si global counter drives scs slots

    pending_norm = None
    for w in range(NW):
        for hp in range(4):
            fq.drain_due(slot(w, hp))
            jv = None
            if do_qkv and hp == 0 and w == 0:
                jv = [vp_unit(j) for j in range(4)]
            pending_norm = attn_pair_window(
                w, hp, fq, pending_norm, jv,
                steps_done, total_steps)
    if pending_norm is not None:
        pending_norm[0]()
        pending_norm[1]()
    while fq.cursor < len(fq.items):
        fq.items[fq.cursor][1]()
        fq.cursor += 1
    if do_proj:
        for u in projs(4 * (NW - 1), 4 * NW):
            u()


def _prep_core_inputs(core, inp, w_attn, b_attn, w_proj):
    b, g = core // 2, core % 2
    qc, kc, vc = 512 * g, D + 512 * g, 2 * D + 512 * g
    return dict(
        inpT=np.ascontiguousarray(inp[b].T).astype(bf16),
        wqk=np.concatenate(
            [w_attn[:, qc:qc+512], w_attn[:, kc:kc+512]], axis=1).astype(bf16),
        wv=w_attn[:, vc:vc+512].astype(bf16),
        wproj=np.ascontiguousarray(w_proj[512*g:512*(g+1), :]).astype(bf16),
        bqk=np.concatenate([b_attn[qc:qc+512], b_attn[kc:kc+512]])
            .astype(np.float32).reshape(8, 128).T.copy(),
        bv=b_attn[vc:vc+512].astype(bf16).reshape(1, 512),
        trimask=np.triu(np.ones((128, 128), np.float32)).astype(bf16),
    )


def kernel(inp, w_attn, b_attn, w_proj, b_proj, _results_out=None):
    inp = np.asarray(inp, dtype=np.float32)
    w_attn = np.asarray(w_attn, dtype=np.float32)
    b_attn = np.asarray(b_attn, dtype=np.float32)
    w_proj = np.asarray(w_proj, dtype=np.float32)
    b_proj = np.asarray(b_proj, dtype=np.float32)

    with_bias = bool(np.any(b_attn != 0.0))
    key = (1, with_bias)
    if key not in _NC_CACHE:
        _NC_CACHE[key] = build_nc(reps=1, with_bias=with_bias)
    nc = _NC_CACHE[key]

    in_maps = [_prep_core_inputs(c, inp, w_attn, b_attn, w_proj)
               for c in range(NCORE)]
    declared = set()
    for alloc in nc.m.functions[0].allocations:
        if isinstance(alloc, mybir.MemoryLocationSet) and alloc.kind == "ExternalInput":
            declared.add(alloc.memorylocations[0].name)
    in_maps = [{k: v for k, v in m.items() if k in declared} for m in in_maps]

    res = run_bass_kernel_spmd(nc, in_maps, core_ids=list(range(NCORE)))
    if _results_out is not None:
        _results_out.append(res)

    out = np.empty((B, S, D), np.float32)
    for b in range(B):
        out[b] = (res.results[2*b]["out"] + res.results[2*b+1]["out"]
                  + b_proj[None, :])
    return out

